# revision 1
# baseline (speedup 1.0000x reference)
"""DSS (Diagonal State Space) layer as a Bass/Tile kernel for 8 Trainium2 NeuronCores.

Algorithm (per core, channels H sharded 8 x 128):
  1. Build the DSS-exp kernel k[l,h] = Re(sum_n W[h,n] z[h,n]^l), z = exp(dt_h * Lambda_n),
     on-device via a two-level power factorization l = 32a + b:
       GW[h,n,b] = W * z^b (b<32),  Z32[h,n,a] = z^(32a) (a<16), both by complex doubling,
     then a per-channel PE matmul contracts the 64 modes (re/im packed into 128 partitions).
  2. K_f = rfft_1024(k) via PE matmuls against host-precomputed DFT tiles.
  3. Overlap-save FFT convolution: per 512-sample block, forward rfft-1024 as PE matmuls
     (packed 512-frequency layout, Nyquist folded into the sin-tile f=0 slot), complex
     pointwise multiply split across DVE/GPSIMD, inverse rfft as PE matmuls producing the
     valid 512 samples.
  4. The skip connection y += u * D is folded into the frequency-domain filter
     (K'_f = K_f + D), so it costs nothing in the main loop.

All matmuls use float32r (full PE rate; ~1.6e-4 relative rounding). Transcendentals are
evaluated with small-argument polynomials on DVE (the ACT LUTs are only ~1e-4 accurate,
which would compound through the z^511 power chains). The forward runs one block ahead
of the inverse in the PE stream; DMA traffic is spread across the SP/ACT HWDGE queues.
"""

import sys

for _p in ("/opt/trn_rl_repo", "/opt/trn_rl_repo/concourse"):
    if _p not in sys.path:
        sys.path.insert(0, _p)

import numpy as np
from contextlib import ExitStack

import concourse.bacc as bacc
import concourse.tile as tile
import concourse.mybir as mybir

dt = mybir.dt
f32 = np.float32

B, L, H, N = 4, 4096, 1024, 64
LK = 512
F = 1024          # FFT length (overlap-save)
HOP = 512         # block hop
NCORES = 8
HS = H // NCORES  # 128 channels per core
NBLK = L // HOP   # 8
NFT = 4           # packed frequency tiles (512 freqs + Nyquist folded)
NJ = F // 128     # 8 contraction chunks for the forward DFT
NLT = HOP // 128  # 4 output l-tiles per block
NCH = L // 128    # 32 u chunks per core


# ---------------------------------------------------------------- host constants
def build_constants():
    l = np.arange(F, dtype=np.float64)[:, None]
    f = np.arange(512, dtype=np.float64)[None, :]
    ang = 2 * np.pi * l * f / F
    C = np.cos(ang)
    S = -np.sin(ang)
    S[:, 0] = (-1.0) ** np.arange(F)      # Nyquist row packed into sin-tile col 0
    CF = np.zeros((NJ, 2, NFT, 128, 128))
    for j in range(NJ):
        for ft in range(NFT):
            CF[j, 0, ft] = C[128 * j:128 * j + 128, 128 * ft:128 * ft + 128]
            CF[j, 1, ft] = S[128 * j:128 * j + 128, 128 * ft:128 * ft + 128]
    lc = 512 + np.arange(512, dtype=np.float64)[None, :]   # valid circular outputs
    fr = np.arange(512, dtype=np.float64)[:, None]
    cf_ = np.where(fr == 0, 1.0, 2.0)
    Ar = cf_ * np.cos(2 * np.pi * fr * lc / F) / F
    Ai = -(2.0 / F) * np.sin(2 * np.pi * fr * lc / F)
    Ai[0, :] = ((-1.0) ** lc[0]) / F                        # Nyquist inverse row
    AI = np.zeros((2, NFT, NLT, 128, 128))
    for ft in range(NFT):
        for lt in range(NLT):
            AI[0, ft, lt] = Ar[128 * ft:128 * ft + 128, 128 * lt:128 * lt + 128]
            AI[1, ft, lt] = Ai[128 * ft:128 * ft + 128, 128 * lt:128 * lt + 128]
    return CF.astype(f32), AI.astype(f32)


# Horner coefficient lists (highest degree first)
def _fact(k):
    r = 1.0
    for i in range(2, k + 1):
        r *= i
    return r


EXP10 = [1.0 / _fact(k) for k in range(10, -1, -1)]          # e^x, |x| <~ 0.9
EXP9 = [1.0 / _fact(k) for k in range(9, -1, -1)]            # e^x, |x| <~ 0.4
SIN9 = [1.0 / _fact(9), -1.0 / _fact(7), 1.0 / _fact(5), -1.0 / _fact(3), 1.0]   # odd, in u = x^2
COSC = [1.0 / _fact(10), -1.0 / _fact(8), 1.0 / _fact(6), -1.0 / _fact(4),
        1.0 / _fact(2)]          # cos(x) = 1 - u*POLY? see _cos_poly


class _Prog:
    def __init__(self):
        self.nc = None
        self.built = False


_prog = _Prog()


def _emit_kernel(nc, tc, ctx, aps):
    V = nc.vector
    A = nc.scalar
    T = nc.tensor
    u_ap = aps["u"]; y_ap = aps["y"]
    cf_ap = aps["CF"]; ai_ap = aps["AI"]
    TT = V.tensor_tensor
    GT = nc.gpsimd.tensor_tensor
    op = mybir.AluOpType

    # ---------------- pools
    p_cf = ctx.enter_context(tc.tile_pool(name="cf", bufs=1))
    p_ai = ctx.enter_context(tc.tile_pool(name="ai", bufs=1))
    p_uch = ctx.enter_context(tc.tile_pool(name="uch", bufs=8))
    p_yf = ctx.enter_context(tc.tile_pool(name="yf", bufs=8))
    p_krep = ctx.enter_context(tc.tile_pool(name="krep", bufs=1))
    p_tmp = ctx.enter_context(tc.tile_pool(name="tmp", bufs=2))
    p_yout = ctx.enter_context(tc.tile_pool(name="yout", bufs=2))
    p_kc = ctx.enter_context(tc.tile_pool(name="kc", bufs=4))
    p_gw = ctx.enter_context(tc.tile_pool(name="gw", bufs=1))
    p_z32 = ctx.enter_context(tc.tile_pool(name="z32", bufs=1))
    p_zp = ctx.enter_context(tc.tile_pool(name="zp", bufs=9))
    p_small = ctx.enter_context(tc.tile_pool(name="small", bufs=1))
    p_gwtmp = ctx.enter_context(tc.tile_pool(name="gwtmp", bufs=1))
    p_drep = ctx.enter_context(tc.tile_pool(name="drep", bufs=1))
    p_ps = ctx.enter_context(tc.tile_pool(name="ps", bufs=6, space="PSUM"))
    p_psk = ctx.enter_context(tc.tile_pool(name="psk", bufs=2, space="PSUM"))

    def fview(t):
        return t[:].bitcast(dt.float32)

    # ---------------- small parameter tiles first (they gate the whole k prologue)
    logdt = p_small.tile([1, HS], dt.float32, tag="logdt")
    A.dma_start(logdt[:], aps["logdt"][:])
    Lre = p_small.tile([1, N], dt.float32, tag="lre")
    A.dma_start(Lre[:], aps["Lre"][:])
    Lim_r = p_small.tile([1, N], dt.float32, tag="lim")
    A.dma_start(Lim_r[:], aps["Lim"][:])

    # ---------------- constant stationaries, merged DMAs spread over both HWDGE queues
    cf_big = {}
    _cfq = [0]

    def load_cf(j, t_):
        tl = p_cf.tile([128, 4, 128], dt.float32r, tag=f"cf{j}_{t_}", name=f"cfb{j}_{t_}")
        eng = (nc.sync, nc.scalar)[_cfq[0] % 2]
        _cfq[0] += 1
        eng.dma_start(tl[:], cf_ap[j, t_].transpose([1, 0, 2]))
        cf_big[(j, t_)] = tl

    for j in (4, 5, 6, 7):
        for t_ in range(2):
            load_cf(j, t_)

    def cf_tile(j, t_, ft):
        return cf_big[(j, t_)][:, ft, :]


    # u chunks: prefetch the first window up front so the in-order DMA queues
    # don't head-of-line block them behind the k-prologue traffic
    chunks = {}

    def get_chunk(c):
        assert c >= 0
        if c not in chunks:
            t_u = p_uch.tile([128, 4, 128], dt.float32r, tag="uch", name=f"uch{c}")
            eng = nc.sync if c % 2 == 0 else nc.scalar
            eng.dma_start(t_u[:], u_ap[:, 128 * c:128 * c + 128, :].transpose([1, 0, 2]))
            chunks[c] = t_u
        return chunks[c]

    for c in range(8):
        get_chunk(c)

    for j in (0, 1, 2, 3):
        for t_ in range(2):
            load_cf(j, t_)

    # inverse stationaries load after the forward-critical traffic
    ai_big = {}
    for t_ in range(2):
        for ft in range(NFT):
            tl = p_ai.tile([128, 4, 128], dt.float32r, tag=f"ai{t_}_{ft}", name=f"aib{t_}_{ft}")
            eng = nc.sync if (ft + t_) % 2 == 0 else nc.scalar
            eng.dma_start(tl[:], ai_ap[t_, ft].transpose([1, 0, 2]))
            ai_big[(t_, ft)] = tl

    def ai_tile(t_, ft, lt):
        return ai_big[(t_, ft)][:, lt, :]

    def emit_fwd(blk):
        out = []
        for ft in range(NFT):
            pc = p_ps.tile([128, 512], dt.float32, tag="ps", name=f"pc{blk}_{ft}")
            psn = p_ps.tile([128, 512], dt.float32, tag="ps", name=f"psn{blk}_{ft}")
            first = True
            for j in range(NJ):
                c = 4 * blk - 4 + j
                if c < 0:
                    continue
                ch = get_chunk(c)
                T.matmul(pc[:], cf_tile(j, 0, ft), ch[:].rearrange("p b h -> p (b h)"),
                         start=first, stop=(j == NJ - 1))
                T.matmul(psn[:], cf_tile(j, 1, ft), ch[:].rearrange("p b h -> p (b h)"),
                         start=first, stop=(j == NJ - 1))
                first = False
            # evacuate psum on ACT so the DVE/GPS pointwise runs all-SBUF
            uc = p_tmp.tile([128, 512], dt.float32, tag="uc", bufs=7, name=f"uc{blk}_{ft}")
            A.copy(uc[:], pc[:])
            us = p_tmp.tile([128, 512], dt.float32, tag="us", bufs=7, name=f"us{blk}_{ft}")
            A.copy(us[:], psn[:])
            out.append((uc, us))
        return out

    fwd_done = {0: emit_fwd(0), 1: emit_fwd(1)}
    PREFETCH3 = True

    def horner_exp(dst, x, coefs):
        # dst = e^x via Horner in x; dst/x are [p, w] fp32 APs; uses p_small temps
        p = dst
        V.memset(p, float(coefs[0]))
        for c in coefs[1:]:
            tq = p_small.tile([x.shape[0], x.shape[1]], dt.float32, tag="horner", bufs=2)
            TT(tq[:], p, x, op.mult)
            V.tensor_scalar_add(p, tq[:], float(c))

    # dt = exp(logdt) = (exp(logdt/8))^8
    x8 = p_small.tile([1, HS], dt.float32, tag="x8")
    V.tensor_scalar_mul(x8[:], logdt[:], 0.125)
    e8 = p_small.tile([1, HS], dt.float32, tag="e8")
    horner_exp(e8[:], x8[:], EXP10)
    dtv = p_small.tile([1, HS], dt.float32, tag="dtv")
    t_a = p_small.tile([1, HS], dt.float32, tag="sq1")
    TT(t_a[:], e8[:], e8[:], op.mult)
    t_b = p_small.tile([1, HS], dt.float32, tag="sq2")
    TT(t_b[:], t_a[:], t_a[:], op.mult)
    TT(dtv[:], t_b[:], t_b[:], op.mult)

    # -exp(Lre) = -(exp(Lre/8))^8
    xl = p_small.tile([1, N], dt.float32, tag="xl")
    V.tensor_scalar_mul(xl[:], Lre[:], 0.125)
    el8 = p_small.tile([1, N], dt.float32, tag="el8")
    horner_exp(el8[:], xl[:], EXP10)
    t_c = p_small.tile([1, N], dt.float32, tag="sq3")
    TT(t_c[:], el8[:], el8[:], op.mult)
    t_d = p_small.tile([1, N], dt.float32, tag="sq4")
    TT(t_d[:], t_c[:], t_c[:], op.mult)
    negel = p_small.tile([1, N], dt.float32, tag="negel")
    t_e = p_small.tile([1, N], dt.float32, tag="sq5")
    TT(t_e[:], t_d[:], t_d[:], op.mult)
    V.tensor_scalar_mul(negel[:], t_e[:], -1.0)

    # outer products: a[n,h] = -e^{Lre_n} dt_h ; b[n,h] = Lim_n dt_h
    ps_a = p_psk.tile([128, 512], dt.float32, tag="psk")
    T.matmul(ps_a[0:N, 0:HS], negel[:], dtv[:], start=True, stop=True)
    ps_b = p_psk.tile([128, 512], dt.float32, tag="psk")
    T.matmul(ps_b[0:N, 0:HS], Lim_r[:], dtv[:], start=True, stop=True)

    # half-angle pieces on [N, HS]
    ah = p_small.tile([N, HS], dt.float32, tag="ah")
    V.tensor_scalar_mul(ah[:], ps_a[0:N, 0:HS], 0.5)
    bh = p_small.tile([N, HS], dt.float32, tag="bh")
    V.tensor_scalar_mul(bh[:], ps_b[0:N, 0:HS], 0.5)
    ea = p_small.tile([N, HS], dt.float32, tag="ea")
    horner_exp(ea[:], ah[:], EXP9)
    # sin(bh), cos(bh) via u = bh^2
    ub = p_small.tile([N, HS], dt.float32, tag="ub")
    TT(ub[:], bh[:], bh[:], op.mult)
    sp = p_small.tile([N, HS], dt.float32, tag="sp")
    V.memset(sp[:], float(SIN9[0]))
    for c in SIN9[1:]:
        tq = p_small.tile([N, HS], dt.float32, tag="horner", bufs=2)
        TT(tq[:], sp[:], ub[:], op.mult)
        V.tensor_scalar_add(sp[:], tq[:], float(c))
    sb = p_small.tile([N, HS], dt.float32, tag="sb")
    TT(sb[:], sp[:], bh[:], op.mult)          # sin(b/2)
    cp = p_small.tile([N, HS], dt.float32, tag="cp")
    V.memset(cp[:], float(COSC[0]))
    for c in COSC[1:]:
        tq = p_small.tile([N, HS], dt.float32, tag="horner", bufs=2)
        TT(tq[:], cp[:], ub[:], op.mult)
        V.tensor_scalar_add(cp[:], tq[:], float(c))
    # cos(x) = 1 - u * cp  (cp = 1/2 - u/24 + ... evaluated via Horner above)
    cb = p_small.tile([N, HS], dt.float32, tag="cb")
    tq = p_small.tile([N, HS], dt.float32, tag="horner", bufs=2)
    TT(tq[:], cp[:], ub[:], op.mult)
    V.tensor_scalar(cb[:], tq[:], -1.0, 1.0, op.mult, op.add)

    wre = p_small.tile([N, HS], dt.float32, tag="wre")
    TT(wre[:], ea[:], cb[:], op.mult)
    wim = p_small.tile([N, HS], dt.float32, tag="wim")
    TT(wim[:], ea[:], sb[:], op.mult)

    # complex squaring on separate re/im planes (all base-partition 0, lane-aligned)
    def csq_parts(dre, dim_, sre, sim):
        t1 = p_small.tile([N, HS], dt.float32, tag="csq1", bufs=2)
        TT(t1[:], sre, sre, op.mult)
        t2 = p_small.tile([N, HS], dt.float32, tag="csq2", bufs=2)
        TT(t2[:], sim, sim, op.mult)
        TT(dre, t1[:], t2[:], op.subtract)
        t3 = p_small.tile([N, HS], dt.float32, tag="csq3", bufs=2)
        TT(t3[:], sre, sim, op.mult)
        V.tensor_scalar_mul(dim_, t3[:], 2.0)

    def new_zpair(nm):
        zr = p_zp.tile([N, HS], dt.float32, tag="zp", name=f"{nm}r")
        zi = p_zp.tile([N, HS], dt.float32, tag="zp", name=f"{nm}i")
        return zr, zi


    # ---------------- GW planes [N, HS, 32] holding (Re, -Im) of W z^b
    GWre_r = p_gw.tile([N, HS, 32], dt.float32r, tag="gwre")
    GWim_r = p_gw.tile([N, HS, 32], dt.float32r, tag="gwim")   # stores -Im
    GWre = GWre_r[:]
    GWim = GWim_r[:]
    A.dma_start(GWre[:, :, 0], aps["Wre"][:].bitcast(dt.float32r))
    wimt = p_small.tile([N, HS], dt.float32, tag="wimt")
    A.dma_start(wimt[:], aps["Wim"][:])
    V.tensor_scalar_mul(GWim[:, :, 0], wimt[:], -1.0)

    def cdouble_seg(pre, pim, zr, zi, s0, d0, w, conj_stored):
        # planes [.., d0:d0+w] = planes[.., s0:s0+w] * (zr + i zi);
        # when conj_stored, the im plane holds the negated imaginary part.
        zre = zr[:].unsqueeze(2).broadcast_to([N, HS, w])
        zim = zi[:].unsqueeze(2).broadcast_to([N, HS, w])
        t2 = p_gwtmp.tile([N, HS, 8], dt.float32, tag="gt2", bufs=3)
        t4 = p_gwtmp.tile([N, HS, 8], dt.float32, tag="gt2", bufs=3)
        TT(pre[:, :, d0:d0 + w], pre[:, :, s0:s0 + w], zre, op.mult)
        GT(t2[:, :, 0:w], pim[:, :, s0:s0 + w], zim, op.mult)
        TT(pim[:, :, d0:d0 + w], pim[:, :, s0:s0 + w], zre, op.mult)
        GT(t4[:, :, 0:w], pre[:, :, s0:s0 + w], zim, op.mult)
        TT(pre[:, :, d0:d0 + w], pre[:, :, d0:d0 + w], t2[:, :, 0:w],
           op.add if conj_stored else op.subtract)
        TT(pim[:, :, d0:d0 + w], pim[:, :, d0:d0 + w], t4[:, :, 0:w],
           op.subtract if conj_stored else op.add)

    def cdouble(pre, pim, zr, zi, w, conj_stored):
        cdouble_seg(pre, pim, zr, zi, 0, w, w, conj_stored)


    # ---------------- Z32 planes [N, HS, 16] natural complex z^(32a)
    Zre_r = p_z32.tile([N, HS, 16], dt.float32r, tag="z32re")
    Zim_r = p_z32.tile([N, HS, 16], dt.float32r, tag="z32im")
    Zre = Zre_r[:]
    Zim = Zim_r[:]
    # a=0 plane is the complex constant 1+0i (memset can't emit float32r)
    V.tensor_scalar(Zre[:, :, 0], wre[:], 0.0, 1.0, op.mult, op.add)
    V.tensor_scalar(Zim[:, :, 0], wre[:], 0.0, 0.0, op.mult, op.add)

    # interleaved power chain + doubling: GW level j follows zp[j] immediately,
    # Z32 level j follows za[j], keeping the serial latency to a minimum
    zp = []
    z0 = new_zpair("z0")
    csq_parts(z0[0][:], z0[1][:], wre[:], wim[:])
    zp.append(z0)
    cdouble(GWre, GWim, zp[0][0], zp[0][1], 1, conj_stored=True)
    for j in range(1, 5):                     # z^2, z^4, z^8, z^16
        zj = new_zpair(f"z{1 << j}")
        csq_parts(zj[0][:], zj[1][:], zp[-1][0][:], zp[-1][1][:])
        zp.append(zj)
        if j < 4:
            cdouble(GWre, GWim, zp[j][0], zp[j][1], 1 << j, conj_stored=True)
    za = []
    z32t = new_zpair("z32")
    csq_parts(z32t[0][:], z32t[1][:], zp[4][0][:], zp[4][1][:])
    za.append(z32t)                           # z^32
    cdouble_seg(GWre, GWim, zp[4][0], zp[4][1], 0, 16, 8, conj_stored=True)
    cdouble_seg(GWre, GWim, zp[4][0], zp[4][1], 8, 24, 8, conj_stored=True)
    cdouble(Zre, Zim, za[0][0], za[0][1], 1, conj_stored=False)
    for j in range(1, 4):                     # z^64, z^128, z^256
        zj = new_zpair(f"za{j}")
        csq_parts(zj[0][:], zj[1][:], za[-1][0][:], za[-1][1][:])
        za.append(zj)
        cdouble(Zre, Zim, za[j][0], za[j][1], 1 << j, conj_stored=False)

    # ---------------- mode-sum: k[32a+b, h], two contraction-64 matmuls per channel
    # psum += GWre_h^T @ Zre_h ; psum += GWim_h^T @ Zim_h  (im plane is negated)
    ks = []
    for g in range(4):
        kp_g = p_psk.tile([32, 32, 16], dt.float32, tag="psk", name=f"kp{g}")
        for hl in range(32):
            h = 32 * g + hl
            T.matmul(kp_g[0:32, hl, :], GWre_r[:, h, :], Zre_r[:, h, :],
                     start=True, stop=False)
            T.matmul(kp_g[0:32, hl, :], GWim_r[:, h, :], Zim_r[:, h, :],
                     start=False, stop=True)
        # evacuate lane-aligned with a-major free order (contiguous shuffle reads)
        t_ks = p_yout.tile([32, 16, 32], dt.float32r, tag="yout", name=f"ks{g}")
        A.copy(t_ks[:], kp_g[:].transpose([0, 2, 1]))
        ks.append(t_ks)
    kc = []
    for c in range(4):
        kc.append(p_kc.tile([128, 128], dt.float32r, tag="kc", name=f"kc{c}"))
    kqi = 0
    for g in range(4):
        for c in range(4):
            for al in range(4):
                eng = (nc.sync, nc.scalar, nc.gpsimd)[kqi % 3]
                kqi += 1
                eng.dma_start(kc[c][:][32 * al:32 * al + 32, 32 * g:32 * g + 32],
                              ks[g][0:32, 4 * c + al, :])

    # ---------------- D_rep [128, 128] (D broadcast down partitions; folded into K)
    dtile = p_small.tile([1, HS], dt.float32, tag="dtile")
    A.dma_start(dtile[:], aps["D"][:])
    ones = p_small.tile([1, 128], dt.float32, tag="ones")
    V.memset(ones[:], 1.0)
    ps_d = p_psk.tile([128, 512], dt.float32, tag="psk")
    T.matmul(ps_d[0:128, 0:HS], ones[:], dtile[:], start=True, stop=True)
    D_rep = p_drep.tile([128, 128], dt.float32, tag="drep")
    A.copy(D_rep[:], ps_d[0:128, 0:HS])

    # ---------------- K_f via packed DFT (reuse forward stationaries j=0..3)
    kdft_ps = {}
    pks = {}
    for t_ in range(2):
        pks[t_] = p_psk.tile([128, 4, 128], dt.float32, tag="psk", name=f"kdft{t_}")
    for ft in range(NFT):
        for t_ in range(2):
            for c in range(4):
                T.matmul(pks[t_][:, ft, :], cf_tile(c, t_, ft), kc[c][:],
                         start=(c == 0), stop=(c == 3))
            kdft_ps[(t_, ft)] = pks[t_][:, ft, :]

    # Krep tensors [128, 128]; the pointwise broadcasts them across the 4 batch groups
    zrow = p_small.tile([1, 128], dt.float32, tag="zrow")
    V.memset(zrow[:], 0.0)
    # the skip connection u*D folds into the filter: K'_f = K_f + D (real part, all f)
    krA, krBC = [], []
    for ft in range(NFT):
        ta = p_krep.tile([128, 128], dt.float32r, tag=f"krA{ft}")
        tb = p_krep.tile([128, 128], dt.float32r, tag=f"krB{ft}")
        TT(ta[:], kdft_ps[(0, ft)], D_rep[:], op.add)
        A.copy(tb[:], kdft_ps[(1, ft)])
        krA.append(ta)
        krBC.append(tb)
    krD0 = p_krep.tile([128, 128], dt.float32r, tag="krD0")
    TT(krD0[:], kdft_ps[(0, 0)], D_rep[:], op.add)
    # row 0 of D-tensor holds K512r (from the packed sin psum row 0), plus D
    TT(krD0[0:1, :], kdft_ps[(1, 0)][0:1, :], D_rep[0:1, :], op.add)
    V.tensor_scalar(krBC[0][0:1, :], zrow[:], 0.0, 0.0, op.mult, op.add)     # Ki slot for f=0/Nyquist is zero

    # ---------------- D_rep [128, 512]
    dtile = p_small.tile([1, HS], dt.float32, tag="dtile")
    A.dma_start(dtile[:], aps["D"][:])
    ones = p_small.tile([1, 128], dt.float32, tag="ones")
    V.memset(ones[:], 1.0)
    ps_d = p_psk.tile([128, 512], dt.float32, tag="psk")
    T.matmul(ps_d[0:128, 0:HS], ones[:], dtile[:], start=True, stop=True)
    D_rep = p_drep.tile([128, 128], dt.float32, tag="drep")
    A.copy(D_rep[:], ps_d[0:128, 0:HS])

    # ---------------- main loop: overlap-save blocks

    def kb(t):
        return t[:].unsqueeze(1).broadcast_to([128, 4, 128])

    fwd_done[2] = emit_fwd(2)

    for blk in range(NBLK):
        yr_t, yi_t = [], []
        fwd = fwd_done.pop(blk)
        if blk + 1 < NBLK and blk + 1 not in fwd_done:
            fwd_done[blk + 1] = emit_fwd(blk + 1)
        for ft in range(NFT):
            uc, us = fwd[ft]
            uc3 = uc[:].rearrange("p (b h) -> p b h", b=4)
            us3 = us[:].rearrange("p (b h) -> p b h", b=4)

            # pointwise: Yr = Uc*A - Us*BC ; Yi = Uc*BC + Us*D
            dten = krD0 if ft == 0 else krA[ft]
            t1 = p_tmp.tile([128, 512], dt.float32, tag="t1")
            t2 = p_tmp.tile([128, 512], dt.float32, tag="t2")
            TT(t1[:].rearrange("p (b h) -> p b h", b=4), uc3, kb(krA[ft]), op.mult)
            GT(t2[:].rearrange("p (b h) -> p b h", b=4), us3, kb(krBC[ft]), op.mult)
            yr = p_yf.tile([128, 512], dt.float32r, tag="yf")
            TT(yr[:], t1[:], t2[:], op.subtract)
            t3 = p_tmp.tile([128, 512], dt.float32, tag="t1")
            t4 = p_tmp.tile([128, 512], dt.float32, tag="t2")
            GT(t3[:].rearrange("p (b h) -> p b h", b=4), uc3, kb(krBC[ft]), op.mult)
            TT(t4[:].rearrange("p (b h) -> p b h", b=4), us3, kb(dten), op.mult)
            yi = p_yf.tile([128, 512], dt.float32r, tag="yf")
            TT(yi[:], t3[:], t4[:], op.add)
            yr_t.append(yr)
            yi_t.append(yi)
        for lt in range(NLT):
            py = p_ps.tile([128, 512], dt.float32, tag="ps")
            for ft in range(NFT):
                T.matmul(py[:], ai_tile(0, ft, lt), yr_t[ft][:],
                         start=(ft == 0), stop=False)
                T.matmul(py[:], ai_tile(1, ft, lt), yi_t[ft][:],
                         start=False, stop=(ft == NFT - 1))
            c_out = 4 * blk + lt
            yo = p_yout.tile([128, 512], dt.float32, tag="yout")
            A.copy(yo[:], py[:])
            eng = nc.sync if lt % 2 == 0 else nc.scalar
            eng.dma_start(y_ap[:, 128 * c_out:128 * c_out + 128, :].transpose([1, 0, 2]),
                          yo[:].rearrange("p (b h) -> p b h", b=4))


def _build_program():
    if _prog.built:
        return
    nc = bacc.Bacc("TRN2", target_bir_lowering=False, debug=False,
                   num_devices=NCORES)
    aps = {}
    aps["u"] = nc.dram_tensor("u", [B, L, HS], dt.float32r, kind="ExternalInput").ap()
    aps["D"] = nc.dram_tensor("D", [1, HS], dt.float32, kind="ExternalInput").ap()
    aps["logdt"] = nc.dram_tensor("logdt", [1, HS], dt.float32, kind="ExternalInput").ap()
    aps["Wre"] = nc.dram_tensor("Wre", [N, HS], dt.float32, kind="ExternalInput").ap()
    aps["Wim"] = nc.dram_tensor("Wim", [N, HS], dt.float32, kind="ExternalInput").ap()
    aps["Lre"] = nc.dram_tensor("Lre", [1, N], dt.float32, kind="ExternalInput").ap()
    aps["Lim"] = nc.dram_tensor("Lim", [1, N], dt.float32, kind="ExternalInput").ap()
    aps["CF"] = nc.dram_tensor("CF", [NJ, 2, NFT, 128, 128], dt.float32r,
                               kind="ExternalInput").ap()
    aps["AI"] = nc.dram_tensor("AI", [2, NFT, NLT, 128, 128], dt.float32r,
                               kind="ExternalInput").ap()
    aps["y"] = nc.dram_tensor("y", [B, L, HS], dt.float32, kind="ExternalOutput").ap()
    with tile.TileContext(nc, trace_sim=False) as tc:
        with ExitStack() as ctx:
            _emit_kernel(nc, tc, ctx, aps)
    nc.compile()
    _prog.nc = nc
    _prog.CF, _prog.AI = build_constants()
    _prog.built = True


def make_in_maps(u, D, log_dt, W_re, W_im, Lambda_re, Lambda_im):
    _build_program()
    in_maps = []
    for c in range(NCORES):
        h0 = c * HS
        in_maps.append({
            "u": np.ascontiguousarray(u[:, :, h0:h0 + HS], dtype=f32),
            "D": np.ascontiguousarray(D[h0:h0 + HS], dtype=f32).reshape(1, HS),
            "logdt": np.ascontiguousarray(log_dt[h0:h0 + HS], dtype=f32).reshape(1, HS),
            "Wre": np.ascontiguousarray(W_re[h0:h0 + HS].T, dtype=f32),
            "Wim": np.ascontiguousarray(W_im[h0:h0 + HS].T, dtype=f32),
            "Lre": np.ascontiguousarray(Lambda_re, dtype=f32).reshape(1, N),
            "Lim": np.ascontiguousarray(Lambda_im, dtype=f32).reshape(1, N),
            "CF": _prog.CF,
            "AI": _prog.AI,
        })
    return in_maps


LAST_RESULTS = None


def kernel(u, D, Lambda_re, Lambda_im, log_dt, W_re, W_im):
    global LAST_RESULTS
    from concourse.bass_utils import run_bass_kernel_spmd
    in_maps = make_in_maps(u, D, log_dt, W_re, W_im, Lambda_re, Lambda_im)
    res = run_bass_kernel_spmd(_prog.nc, in_maps, core_ids=list(range(NCORES)))
    LAST_RESULTS = res
    y = np.concatenate([res.results[c]["y"] for c in range(NCORES)], axis=2)
    return y.astype(np.float32)



# revision 6
# speedup vs baseline: 1.4158x; 1.4158x over previous
"""DSS (Diagonal State Space) layer as a Bass/Tile kernel for 8 Trainium2 NeuronCores.

Channels H sharded 8 x 128. Per core, a polyphase overlap-save FFT convolution:

  1. Forward: each 512-sample chunk c gets ONE packed 1024-point partial DFT
     A^c = CF_{c%2}^T u_c (32 PE matmuls), where CF_odd = CF_even * (-1)^f.
     Each block t then needs only the sum A^{t-1} + A^t: the (-1)^k phase of the
     second polyphase leg is folded into the odd-chunk stationaries, the filter
     parity variant, and nothing else - each u sample is forward-transformed
     once instead of twice (256 vs 480 big matmuls).
  2. Pointwise: Y = F (.) (A^{t-1}+A^t) with F = s(.)K' on even blocks, K' on odd
     (K' = K_f + D, the skip connection folded in). Sums run on GPSIMD, the
     complex multiply on DVE, all in bf16 (2x DVE rate).
  3. Inverse: one shared inverse-DFT stationary set (window [512,1024)) for all
     blocks; 8 matmuls per 128 output samples, moving operand bf16.
  4. DSS kernel k built on device: transcendental chains restacked to
     [128,64] (full partition width, re on DVE / im on GPSIMD), two-level
     power tables GW (W z^b) / Z (z^32a) by complex doubling, channel-PAIRED
     mode-sum matmuls (zero-quadrant moving operand), SEL-matmul transpose to
     kc l-major layout (no SWDGE shuffle), packed K_f DFT reusing the forward
     chunk stationaries in both parities.
"""

import sys

for _p in ("/opt/trn_rl_repo", "/opt/trn_rl_repo/concourse"):
    if _p not in sys.path:
        sys.path.insert(0, _p)

import numpy as np
from contextlib import ExitStack

import concourse.bacc as bacc
import concourse.tile as tile
import concourse.mybir as mybir

dt = mybir.dt
f32 = np.float32

B, L, H, N = 4, 4096, 1024, 64
LK = 512
F = 1024          # FFT length (overlap-save)
HOP = 512         # block hop / chunk size
NCORES = 8
HS = H // NCORES  # 128 channels per core
NBLK = L // HOP   # 8 blocks == 8 chunks
NFT = 4           # packed frequency tiles (512 freqs, Nyquist folded in sin f=0)
NJ = 4            # contraction sub-chunks per 512-sample chunk
NLT = HOP // 128  # 4 output l-tiles per block
NCH = L // 128    # 32 u sub-chunk tiles per core


# ---------------------------------------------------------------- host constants
def build_constants():
    l = np.arange(F, dtype=np.float64)[:, None]
    f = np.arange(512, dtype=np.float64)[None, :]
    ang = 2 * np.pi * l * f / F
    C = np.cos(ang)
    S = -np.sin(ang)
    S[:, 0] = (-1.0) ** np.arange(F)      # Nyquist row packed into sin-tile col 0
    sgn = (-1.0) ** np.arange(512)        # (-1)^f; f=0 (DC + packed Nyquist) -> +1
    # CF[par, cs, j, l', ft, f]: chunk-DFT stationaries (rows l' = 0..511 only)
    CF = np.zeros((2, 2, NJ, 128, NFT, 128))
    for par in range(2):
        for j in range(NJ):
            for ft in range(NFT):
                cs_c = C[128 * j:128 * j + 128, 128 * ft:128 * ft + 128]
                cs_s = S[128 * j:128 * j + 128, 128 * ft:128 * ft + 128]
                if par == 1:
                    sg = sgn[None, 128 * ft:128 * ft + 128]
                    cs_c = cs_c * sg
                    cs_s = cs_s * sg
                CF[par, 0, j, :, ft, :] = cs_c
                CF[par, 1, j, :, ft, :] = cs_s
    # inverse stationaries, window [512, 1024) (shared by all blocks)
    lc = 512 + np.arange(512, dtype=np.float64)[None, :]
    fr = np.arange(512, dtype=np.float64)[:, None]
    cf_ = np.where(fr == 0, 1.0, 2.0)
    Ar = cf_ * np.cos(2 * np.pi * fr * lc / F) / F
    Ai = -(2.0 / F) * np.sin(2 * np.pi * fr * lc / F)
    Ai[0, :] = ((-1.0) ** lc[0]) / F      # Nyquist inverse row
    AI = np.zeros((2, NFT, 128, NLT, 128))
    for ft in range(NFT):
        for lt in range(NLT):
            AI[0, ft, :, lt, :] = Ar[128 * ft:128 * ft + 128, 128 * lt:128 * lt + 128]
            AI[1, ft, :, lt, :] = Ai[128 * ft:128 * ft + 128, 128 * lt:128 * lt + 128]
    # SGN row for the sign-flipped D outer product
    SGN = sgn.reshape(NFT, 128)[0:1, :].copy()   # (-1)^f pattern repeats per ft tile
    # SEL[b, al, l]: SEL[b, al, 32*al+b] = 1  (kc partition placement)
    SEL = np.zeros((32, 4, 128))
    for al in range(4):
        for b in range(32):
            SEL[b, al, 32 * al + b] = 1.0
    return CF.astype(f32), AI.astype(f32), SGN.astype(f32), SEL.astype(f32)


# Horner coefficient lists (highest degree first)
def _fact(k):
    r = 1.0
    for i in range(2, k + 1):
        r *= i
    return r


EXP10 = [1.0 / _fact(k) for k in range(10, -1, -1)]          # e^x, |x| <~ 0.9
EXP9 = [1.0 / _fact(k) for k in range(9, -1, -1)]            # e^x, |x| <~ 0.4
SIN9 = [1.0 / _fact(9), -1.0 / _fact(7), 1.0 / _fact(5), -1.0 / _fact(3), 1.0]
COSC = [1.0 / _fact(10), -1.0 / _fact(8), 1.0 / _fact(6), -1.0 / _fact(4),
        1.0 / _fact(2)]


class _Prog:
    def __init__(self):
        self.nc = None
        self.built = False


_prog = _Prog()


def _emit_kernel(nc, tc, ctx, aps):
    V = nc.vector
    A = nc.scalar
    T = nc.tensor
    G = nc.gpsimd
    u_ap = aps["u"]; y_ap = aps["y"]
    cf_ap = aps["CF"]; ai_ap = aps["AI"]
    TT = V.tensor_tensor
    GT = G.tensor_tensor
    op = mybir.AluOpType

    # ---------------- pools
    p_cf = ctx.enter_context(tc.tile_pool(name="cf", bufs=1))
    p_ai = ctx.enter_context(tc.tile_pool(name="ai", bufs=1))
    p_uch = ctx.enter_context(tc.tile_pool(name="uch", bufs=8))
    p_apl = ctx.enter_context(tc.tile_pool(name="apl", bufs=24))   # A planes bf16
    p_asum = ctx.enter_context(tc.tile_pool(name="asum", bufs=12))  # A sums bf16
    p_yf = ctx.enter_context(tc.tile_pool(name="yf", bufs=12))     # Y tiles bf16
    p_tmp = ctx.enter_context(tc.tile_pool(name="tmp", bufs=6))    # cm temps bf16
    p_flt = ctx.enter_context(tc.tile_pool(name="flt", bufs=1))    # filter tiles
    p_yout = ctx.enter_context(tc.tile_pool(name="yout", bufs=3))
    p_kc = ctx.enter_context(tc.tile_pool(name="kc", bufs=4))
    p_gw = ctx.enter_context(tc.tile_pool(name="gw", bufs=1))
    p_z32 = ctx.enter_context(tc.tile_pool(name="z32", bufs=1))
    p_zp = ctx.enter_context(tc.tile_pool(name="zp", bufs=9))
    p_small = ctx.enter_context(tc.tile_pool(name="small", bufs=1))
    p_gwtmp = ctx.enter_context(tc.tile_pool(name="gwtmp", bufs=1))
    p_ks = ctx.enter_context(tc.tile_pool(name="ks", bufs=1))
    p_ps = ctx.enter_context(tc.tile_pool(name="ps", bufs=6, space="PSUM"))
    p_psk = ctx.enter_context(tc.tile_pool(name="psk", bufs=2, space="PSUM"))

    def fr_(t):
        return t.bitcast(dt.float32r)

    # ---------------- small parameter tiles first (they gate the k prologue)
    logdt = p_small.tile([1, HS], dt.float32, tag="logdt")
    A.dma_start(logdt[:], aps["logdt"][:])
    Lre = p_small.tile([1, N], dt.float32, tag="lre")
    A.dma_start(Lre[:], aps["Lre"][:])
    Lim_r = p_small.tile([1, N], dt.float32, tag="lim")
    A.dma_start(Lim_r[:], aps["Lim"][:])
    dtile = p_small.tile([1, HS], dt.float32, tag="dtile")
    A.dma_start(dtile[:], aps["D"][:])
    sgn_row = p_small.tile([1, 128], dt.float32, tag="sgnrow")
    A.dma_start(sgn_row[:], aps["SGN"][:])
    sel_t = p_small.tile([32, 4, 128], dt.float32, tag="sel")
    A.dma_start(sel_t[:], aps["SEL"][:])
    # restacked W planes [128=(n,hpar), 64]
    wre2 = p_small.tile([128, 64], dt.float32, tag="wre2")
    A.dma_start(wre2[:], aps["Wre2"][:])
    wim2 = p_small.tile([128, 64], dt.float32, tag="wim2")
    A.dma_start(wim2[:], aps["Wim2"][:])

    # ---------------- constant stationaries: CF (both parities), then AI
    cf_big = {}
    _q = [0]

    def qeng():
        e = (nc.sync, nc.scalar)[_q[0] % 2]
        _q[0] += 1
        return e

    def load_cf(par, cs, j):
        tl = p_cf.tile([128, NFT, 128], dt.float32, tag=f"cf{par}{cs}{j}",
                       name=f"cfb{par}_{cs}_{j}")
        qeng().dma_start(tl[:], cf_ap[par, cs, j])
        cf_big[(par, cs, j)] = tl

    for j in range(NJ):
        for cs in range(2):
            load_cf(0, cs, j)

    def cf_tile(par, cs, j, ft):
        return cf_big[(par, cs, j)][:, ft, :]

    # u sub-chunk tiles (128 samples each); chunk c uses tiles 4c..4c+3
    chunks = {}

    def get_chunk(cc):
        if cc not in chunks:
            t_u = p_uch.tile([128, 4, 128], dt.float32r, tag="uch", name=f"uch{cc}")
            qeng().dma_start(t_u[:], u_ap[:, 128 * cc:128 * cc + 128, :].transpose([1, 0, 2]))
            chunks[cc] = t_u
        return chunks[cc]

    for cc in range(8):
        get_chunk(cc)

    for j in range(NJ):
        for cs in range(2):
            load_cf(1, cs, j)

    ai_big = {}
    for cs in range(2):
        for ft in range(NFT):
            tl = p_ai.tile([128, NLT, 128], dt.float32, tag=f"ai{cs}_{ft}",
                           name=f"aib{cs}_{ft}")
            qeng().dma_start(tl[:], ai_ap[cs, ft])
            ai_big[(cs, ft)] = tl

    def ai_tile(cs, ft, lt):
        return ai_big[(cs, ft)][:, lt, :]

    # ---------------- forward chunk-DFT: A^c planes (bf16), 32 matmuls per chunk
    a_planes = {}   # (c, cs, ft) -> bf16 [128, 512]

    def emit_chunk_dft(c):
        par = c % 2
        for ft in range(NFT):
            for cs in range(2):
                ps = p_ps.tile([128, 512], dt.float32, tag="ps", name=f"cdft{c}_{ft}_{cs}")
                for j in range(NJ):
                    ch = get_chunk(4 * c + j)
                    T.matmul(ps[:], fr_(cf_tile(par, cs, j, ft)),
                             ch[:].rearrange("p b h -> p (b h)"),
                             start=(j == 0), stop=(j == NJ - 1))
                ap_t = p_apl.tile([128, 512], dt.bfloat16, tag="apl",
                                  name=f"A{c}_{cs}_{ft}")
                A.copy(ap_t[:], ps[:])
                a_planes[(c, cs, ft)] = ap_t

    # ---------------- transcendental prologue (restacked [128, 64])
    def horner_exp(dst, x, coefs, eng=V):
        p = dst
        eng.memset(p, float(coefs[0]))
        for c_ in coefs[1:]:
            tq = p_small.tile([x.shape[0], x.shape[1]], dt.float32, tag="horner", bufs=2)
            eng.tensor_tensor(tq[:], p, x, op.mult)
            eng.tensor_scalar_add(p, tq[:], float(c_))

    # dt = exp(logdt) = (exp(logdt/8))^8   on [1, HS]
    x8 = p_small.tile([1, HS], dt.float32, tag="x8")
    V.tensor_scalar_mul(x8[:], logdt[:], 0.125)
    e8 = p_small.tile([1, HS], dt.float32, tag="e8")
    horner_exp(e8[:], x8[:], EXP10)
    dtv = p_small.tile([1, HS], dt.float32, tag="dtv")
    t_a = p_small.tile([1, HS], dt.float32, tag="sq1")
    TT(t_a[:], e8[:], e8[:], op.mult)
    t_b = p_small.tile([1, HS], dt.float32, tag="sq2")
    TT(t_b[:], t_a[:], t_a[:], op.mult)
    TT(dtv[:], t_b[:], t_b[:], op.mult)

    # -exp(Lre) on [1, N] (GPS lane, runs parallel to dtv chain)
    xl = p_small.tile([1, N], dt.float32, tag="xl")
    G.tensor_scalar_mul(xl[:], Lre[:], 0.125)
    el8 = p_small.tile([1, N], dt.float32, tag="el8")
    horner_exp(el8[:], xl[:], EXP10, eng=G)
    t_c = p_small.tile([1, N], dt.float32, tag="sq3")
    GT(t_c[:], el8[:], el8[:], op.mult)
    t_d = p_small.tile([1, N], dt.float32, tag="sq4")
    GT(t_d[:], t_c[:], t_c[:], op.mult)
    negel = p_small.tile([1, N], dt.float32, tag="negel")
    t_e = p_small.tile([1, N], dt.float32, tag="sq5")
    GT(t_e[:], t_d[:], t_d[:], op.mult)
    G.tensor_scalar_mul(negel[:], t_e[:], -1.0)

    # ---------------- outer products on PE (partition halves), FIRST PE instrs
    # a2[n+64q, c] = -e^{Lre_n} * dt[2c+q] ; b2[n+64q, c] = Lim_n * dt[2c+q]
    ps_ab = p_psk.tile([128, 512], dt.float32, tag="psk", name="ps_ab")
    for q in range(2):
        dt_half = dtv[:, q::2]
        T.matmul(ps_ab[q * 64:q * 64 + 64, 0:64], negel[:], dt_half, start=True, stop=True)
        T.matmul(ps_ab[q * 64:q * 64 + 64, 64:128], Lim_r[:], dt_half, start=True, stop=True)
    # D_rep / D_rep_s [128, 128]
    ones = p_small.tile([1, 128], dt.float32, tag="ones")
    V.memset(ones[:], 1.0)
    ps_d = p_psk.tile([128, 256], dt.float32, tag="psk", name="ps_d")
    T.matmul(ps_d[0:128, 0:HS], ones[:], dtile[:], start=True, stop=True)
    T.matmul(ps_d[0:128, 128:128 + HS], sgn_row[:], dtile[:], start=True, stop=True)
    D_rep = p_small.tile([128, 128], dt.float32, tag="drep")
    A.copy(D_rep[:], ps_d[0:128, 0:HS])
    D_rep_s = p_small.tile([128, 128], dt.float32, tag="dreps")
    A.copy(D_rep_s[:], ps_d[0:128, 128:128 + HS])

    # ---------------- chunk DFTs 0..2 cover the prologue on PE
    emit_chunk_dft(0)
    emit_chunk_dft(1)
    emit_chunk_dft(2)

    # ---------------- half-angle pieces on [128, 64]
    ah = p_small.tile([128, 64], dt.float32, tag="ah")
    V.tensor_scalar_mul(ah[:], ps_ab[:, 0:64], 0.5)
    bh = p_small.tile([128, 64], dt.float32, tag="bh")
    G.tensor_scalar_mul(bh[:], ps_ab[:, 64:128], 0.5)
    ea = p_small.tile([128, 64], dt.float32, tag="ea")
    horner_exp(ea[:], ah[:], EXP9)
    # sin(bh), cos(bh) via u = bh^2 (sin chain on GPS, cos on DVE)
    ub = p_small.tile([128, 64], dt.float32, tag="ub")
    GT(ub[:], bh[:], bh[:], op.mult)
    sp = p_small.tile([128, 64], dt.float32, tag="sp")
    G.memset(sp[:], float(SIN9[0]))
    for c_ in SIN9[1:]:
        tq = p_small.tile([128, 64], dt.float32, tag="hornerg", bufs=2)
        GT(tq[:], sp[:], ub[:], op.mult)
        G.tensor_scalar_add(sp[:], tq[:], float(c_))
    sb = p_small.tile([128, 64], dt.float32, tag="sb")
    GT(sb[:], sp[:], bh[:], op.mult)          # sin(b/2)
    cp = p_small.tile([128, 64], dt.float32, tag="cp")
    V.memset(cp[:], float(COSC[0]))
    for c_ in COSC[1:]:
        tq = p_small.tile([128, 64], dt.float32, tag="horner", bufs=2)
        TT(tq[:], cp[:], ub[:], op.mult)
        V.tensor_scalar_add(cp[:], tq[:], float(c_))
    cb = p_small.tile([128, 64], dt.float32, tag="cb")
    tq0 = p_small.tile([128, 64], dt.float32, tag="horner", bufs=2)
    TT(tq0[:], cp[:], ub[:], op.mult)
    V.tensor_scalar(cb[:], tq0[:], -1.0, 1.0, op.mult, op.add)   # cos(b/2)

    wre = p_small.tile([128, 64], dt.float32, tag="wre")
    TT(wre[:], ea[:], cb[:], op.mult)         # Re z^(1/2)
    wim = p_small.tile([128, 64], dt.float32, tag="wim")
    GT(wim[:], ea[:], sb[:], op.mult)         # Im z^(1/2)

    # complex squaring: re parts on DVE, im on GPS
    def csq_parts(dre, dim_, sre, sim):
        t1 = p_small.tile([128, 64], dt.float32, tag="csq1", bufs=2)
        TT(t1[:], sre, sre, op.mult)
        t2 = p_small.tile([128, 64], dt.float32, tag="csq2", bufs=2)
        TT(t2[:], sim, sim, op.mult)
        TT(dre, t1[:], t2[:], op.subtract)
        t3 = p_small.tile([128, 64], dt.float32, tag="csq3", bufs=2)
        GT(t3[:], sre, sim, op.mult)
        G.tensor_scalar_mul(dim_, t3[:], 2.0)

    def new_zpair(nm):
        zr = p_zp.tile([128, 64], dt.float32, tag="zp", name=f"{nm}r")
        zi = p_zp.tile([128, 64], dt.float32, tag="zp", name=f"{nm}i")
        return zr, zi

    # ---------------- GW planes [128, 64, 32] holding (Re, -Im) of W z^b
    GWre = p_gw.tile([128, 64, 32], dt.float32, tag="gwre")
    GWim = p_gw.tile([128, 64, 32], dt.float32, tag="gwim")   # stores -Im
    V.tensor_copy(GWre[:, :, 0], wre2[:])
    V.tensor_scalar_mul(GWim[:, :, 0], wim2[:], -1.0)

    def cdouble_seg(pre, pim, zr, zi, s0, d0, w, conj_stored, pr=slice(0, 128), co=0):
        # planes [pr, :, co+d0 : co+d0+w] = planes[pr, :, co+s0:+w] * (zr + i zi)
        nhp = pre.shape[1]
        npr = pr.stop - pr.start
        zre = zr[pr].unsqueeze(2).broadcast_to([npr, nhp, w])
        zim = zi[pr].unsqueeze(2).broadcast_to([npr, nhp, w])
        t2 = p_gwtmp.tile([128, 64, 8], dt.float32, tag="gt2", bufs=3)
        t4 = p_gwtmp.tile([128, 64, 8], dt.float32, tag="gt2", bufs=3)
        TT(pre[pr, :, co + d0:co + d0 + w], pre[pr, :, co + s0:co + s0 + w], zre, op.mult)
        GT(t2[pr, 0:nhp, 0:w], pim[pr, :, co + s0:co + s0 + w], zim, op.mult)
        GT(pim[pr, :, co + d0:co + d0 + w], pim[pr, :, co + s0:co + s0 + w], zre, op.mult)
        TT(t4[pr, 0:nhp, 0:w], pre[pr, :, co + s0:co + s0 + w], zim, op.mult)
        TT(pre[pr, :, co + d0:co + d0 + w], pre[pr, :, co + d0:co + d0 + w],
           t2[pr, 0:nhp, 0:w], op.add if conj_stored else op.subtract)
        GT(pim[pr, :, co + d0:co + d0 + w], pim[pr, :, co + d0:co + d0 + w],
           t4[pr, 0:nhp, 0:w], op.subtract if conj_stored else op.add)

    # ---------------- Z planes [128, 64, 32]: cols 0:16 = even-h z^(32a), zeros;
    # cols 16:32 = zeros, odd-h z^(32a).  (zero quadrants gate the pair mode-sum)
    Zre = p_z32.tile([128, 64, 32], dt.float32, tag="z32re")
    Zim = p_z32.tile([128, 64, 32], dt.float32, tag="z32im")
    G.memset(Zre[0:64, :, 16:32], 0.0)
    G.memset(Zim[0:64, :, 16:32], 0.0)
    G.memset(Zre[64:128, :, 0:16], 0.0)
    G.memset(Zim[64:128, :, 0:16], 0.0)
    V.memset(Zre[0:64, :, 0], 1.0)
    V.memset(Zim[0:64, :, 0], 0.0)
    V.memset(Zre[64:128, :, 16], 1.0)
    V.memset(Zim[64:128, :, 16], 0.0)

    # interleaved power chain + doubling
    zp = []
    z0 = new_zpair("z0")
    csq_parts(z0[0][:], z0[1][:], wre[:], wim[:])
    zp.append(z0)
    cdouble_seg(GWre[:], GWim[:], zp[0][0][:], zp[0][1][:], 0, 1, 1, True)
    for j in range(1, 5):                     # z^2, z^4, z^8, z^16
        zj = new_zpair(f"z{1 << j}")
        csq_parts(zj[0][:], zj[1][:], zp[-1][0][:], zp[-1][1][:])
        zp.append(zj)
        if j < 4:
            cdouble_seg(GWre[:], GWim[:], zp[j][0][:], zp[j][1][:], 0, 1 << j, 1 << j, True)
    za = []
    z32t = new_zpair("z32")
    csq_parts(z32t[0][:], z32t[1][:], zp[4][0][:], zp[4][1][:])
    za.append(z32t)                           # z^32
    cdouble_seg(GWre[:], GWim[:], zp[4][0][:], zp[4][1][:], 0, 16, 8, True)
    cdouble_seg(GWre[:], GWim[:], zp[4][0][:], zp[4][1][:], 8, 24, 8, True)
    # Z chains: even half at col base 0, odd half at col base 16
    for q, pr in ((0, slice(0, 64)), (1, slice(64, 128))):
        co = 16 * q
        cdouble_seg(Zre[:], Zim[:], za[0][0][:], za[0][1][:], 0, 1, 1, False, pr, co)
    for j in range(1, 4):                     # z^64, z^128, z^256
        zj = new_zpair(f"za{j}")
        csq_parts(zj[0][:], zj[1][:], za[-1][0][:], za[-1][1][:])
        za.append(zj)
        for q, pr in ((0, slice(0, 64)), (1, slice(64, 128))):
            co = 16 * q
            cdouble_seg(Zre[:], Zim[:], za[j][0][:], za[j][1][:], 0, 1 << j, 1 << j,
                        False, pr, co)

    # ---------------- channel-paired mode-sum: k[32a+b, h] (128 matmuls)
    # pair hp: stationary GW[:, hp, :] (dense), moving Z[:, hp, :] (zero quadrants)
    # out [32 b, 32 = (16 a-even | 16 a-odd)]
    ks_all = p_ks.tile([32, 64, 32], dt.bfloat16, tag="ksall")
    for qq in range(4):
        kp = p_psk.tile([32, 16, 32], dt.float32, tag="psk", name=f"kp{qq}")
        for i in range(16):
            hp = 16 * qq + i
            T.matmul(kp[0:32, i, :], fr_(GWre[:, hp, :]), fr_(Zre[:, hp, :]),
                     start=True, stop=False)
            T.matmul(kp[0:32, i, :], fr_(GWim[:, hp, :]), fr_(Zim[:, hp, :]),
                     start=False, stop=True)
        A.copy(ks_all[0:32, 16 * qq:16 * qq + 16, :], kp[:])

    # ---------------- kc tiles [128 l, 128 h] via SEL matmuls (l-major layout)
    # ks_view(c2, al)[b, h] = k[128 c2 + 32 al + b, h]
    ks_v = ks_all[:].rearrange("b hp (par x) -> b hp par x", par=2)
    kc = []
    for c2 in range(4):
        kps = p_psk.tile([128, 128], dt.float32, tag="psk", name=f"kcps{c2}")
        for al in range(4):
            mov = ks_v[:, :, :, 4 * c2 + al].rearrange("b hp par -> b (hp par)")
            T.matmul(kps[:], fr_(sel_t[:, al, :]), mov, start=(al == 0), stop=(al == 3))
        kt = p_kc.tile([128, 128], dt.bfloat16, tag="kc", name=f"kc{c2}")
        A.copy(kt[:], kps[:])
        kc.append(kt)

    # ---------------- K_f via packed chunk-DFT of k (both parities)
    # each psum is evacuated to SBUF immediately (avoids psum-pool deadlock
    # between the 4 K_f accumulations and the later filter builds)
    pks = {}
    for par in range(2):
        for cs in range(2):
            pp = p_psk.tile([128, NFT, 128], dt.float32, tag="psk", name=f"kdft{par}{cs}")
            for ft in range(NFT):
                for c2 in range(4):
                    T.matmul(pp[:, ft, :], fr_(cf_tile(par, cs, c2, ft)), kc[c2][:],
                             start=(c2 == 0), stop=(c2 == 3))
            sb = p_flt.tile([128, NFT, 128], dt.float32, tag=f"pks{par}{cs}")
            A.copy(sb[:], pp[:])
            pks[(par, cs)] = sb

    # ---------------- filter tiles (bf16 [128, 128]):
    # variant v=0 (even blocks): K-tilde from pks[par=1] + D_rep_s
    # variant v=1 (odd blocks):  K' from pks[par=0] + D_rep
    fA = {}
    fB = {}
    fD0 = {}
    for v, par, drep in ((0, 1, D_rep_s), (1, 0, D_rep)):
        for ft in range(NFT):
            ta = p_flt.tile([128, 128], dt.bfloat16, tag=f"fA{v}{ft}")
            eng = (V, G)[ft % 2]
            eng.tensor_tensor(ta[:], pks[(par, 0)][:, ft, :], drep[:], op.add)
            tb = p_flt.tile([128, 128], dt.bfloat16, tag=f"fB{v}{ft}")
            A.copy(tb[:], pks[(par, 1)][:, ft, :])
            fA[(v, ft)] = ta
            fB[(v, ft)] = tb
        td = p_flt.tile([128, 128], dt.bfloat16, tag=f"fD{v}")
        eng = (G, V)[v]
        eng.tensor_tensor(td[:], pks[(par, 0)][:, 0, :], drep[:], op.add)
        # row 0: packed Nyquist slot: K_nyq + D (no sign flip: (-1)^512 = +1)
        TT(td[0:1, :], pks[(par, 1)][0:1, 0, :], D_rep[0:1, :], op.add)
        fD0[v] = td
        V.memset(fB[(v, 0)][0:1, :], 0.0)     # Im slot for f=0/Nyquist is zero

    # ---------------- main loop
    def kb(t):
        return t[:].unsqueeze(1).broadcast_to([128, 4, 128])

    def r3(t):
        return t[:].rearrange("p (b h) -> p b h", b=4)

    emit_chunk_dft(3)

    for blk in range(NBLK):
        v = blk % 2
        yr_t, yi_t = [], []
        for ft in range(NFT):
            ac_cur = a_planes[(blk, 0, ft)]
            as_cur = a_planes[(blk, 1, ft)]
            if blk == 0:
                xc, xs = ac_cur, as_cur
            else:
                ac_prev = a_planes[(blk - 1, 0, ft)]
                as_prev = a_planes[(blk - 1, 1, ft)]
                xc = p_asum.tile([128, 512], dt.bfloat16, tag="asum",
                                 name=f"xc{blk}_{ft}")
                GT(xc[:], ac_cur[:], ac_prev[:], op.add)
                xs = p_asum.tile([128, 512], dt.bfloat16, tag="asum",
                                 name=f"xs{blk}_{ft}")
                GT(xs[:], as_cur[:], as_prev[:], op.add)
            # complex multiply by the parity filter (DVE, bf16)
            t1 = p_tmp.tile([128, 512], dt.bfloat16, tag="t1")
            TT(r3(t1), r3(xc), kb(fA[(v, ft)]), op.mult)
            t2 = p_tmp.tile([128, 512], dt.bfloat16, tag="t2")
            TT(r3(t2), r3(xs), kb(fB[(v, ft)]), op.mult)
            yr = p_yf.tile([128, 512], dt.bfloat16, tag="yf", name=f"yr{blk}_{ft}")
            TT(yr[:], t1[:], t2[:], op.subtract)
            t3 = p_tmp.tile([128, 512], dt.bfloat16, tag="t1")
            TT(r3(t3), r3(xc), kb(fB[(v, ft)]), op.mult)
            dten = fD0[v] if ft == 0 else fA[(v, ft)]
            t4 = p_tmp.tile([128, 512], dt.bfloat16, tag="t2")
            TT(r3(t4), r3(xs), kb(dten), op.mult)
            yi = p_yf.tile([128, 512], dt.bfloat16, tag="yf", name=f"yi{blk}_{ft}")
            TT(yi[:], t3[:], t4[:], op.add)
            yr_t.append(yr)
            yi_t.append(yi)
        if blk + 4 < NBLK:
            emit_chunk_dft(blk + 4)
        for lt in range(NLT):
            py = p_ps.tile([128, 512], dt.float32, tag="ps", name=f"py{blk}_{lt}")
            for ft in range(NFT):
                T.matmul(py[:], fr_(ai_tile(0, ft, lt)), yr_t[ft][:],
                         start=(ft == 0), stop=False)
                T.matmul(py[:], fr_(ai_tile(1, ft, lt)), yi_t[ft][:],
                         start=False, stop=(ft == NFT - 1))
            c_out = 4 * blk + lt
            yo = p_yout.tile([128, 512], dt.float32, tag="yout")
            A.copy(yo[:], py[:])
            eng = nc.sync if lt % 2 == 0 else nc.scalar
            eng.dma_start(y_ap[:, 128 * c_out:128 * c_out + 128, :].transpose([1, 0, 2]),
                          yo[:].rearrange("p (b h) -> p b h", b=4))


def _build_program():
    if _prog.built:
        return
    nc = bacc.Bacc("TRN2", target_bir_lowering=False, debug=False,
                   num_devices=NCORES)
    aps = {}
    aps["u"] = nc.dram_tensor("u", [B, L, HS], dt.float32r, kind="ExternalInput").ap()
    aps["D"] = nc.dram_tensor("D", [1, HS], dt.float32, kind="ExternalInput").ap()
    aps["logdt"] = nc.dram_tensor("logdt", [1, HS], dt.float32, kind="ExternalInput").ap()
    aps["Wre2"] = nc.dram_tensor("Wre2", [128, 64], dt.float32, kind="ExternalInput").ap()
    aps["Wim2"] = nc.dram_tensor("Wim2", [128, 64], dt.float32, kind="ExternalInput").ap()
    aps["Lre"] = nc.dram_tensor("Lre", [1, N], dt.float32, kind="ExternalInput").ap()
    aps["Lim"] = nc.dram_tensor("Lim", [1, N], dt.float32, kind="ExternalInput").ap()
    aps["CF"] = nc.dram_tensor("CF", [2, 2, NJ, 128, NFT, 128], dt.float32,
                               kind="ExternalInput").ap()
    aps["AI"] = nc.dram_tensor("AI", [2, NFT, 128, NLT, 128], dt.float32,
                               kind="ExternalInput").ap()
    aps["SGN"] = nc.dram_tensor("SGN", [1, 128], dt.float32, kind="ExternalInput").ap()
    aps["SEL"] = nc.dram_tensor("SEL", [32, 4, 128], dt.float32, kind="ExternalInput").ap()
    aps["y"] = nc.dram_tensor("y", [B, L, HS], dt.float32, kind="ExternalOutput").ap()
    with tile.TileContext(nc, trace_sim=False) as tc:
        with ExitStack() as ctx:
            _emit_kernel(nc, tc, ctx, aps)
    nc.compile()
    _prog.nc = nc
    _prog.CF, _prog.AI, _prog.SGN, _prog.SEL = build_constants()
    _prog.built = True


def make_in_maps(u, D, log_dt, W_re, W_im, Lambda_re, Lambda_im):
    _build_program()
    in_maps = []
    for c in range(NCORES):
        h0 = c * HS
        # restack W [HS, N] -> [128=(n, h%2), 64=h//2]
        wre_s = W_re[h0:h0 + HS]    # [HS, N]
        wim_s = W_im[h0:h0 + HS]
        wre2 = np.empty((128, 64), f32)
        wim2 = np.empty((128, 64), f32)
        for q in range(2):
            wre2[64 * q:64 * q + 64, :] = wre_s[q::2, :].T
            wim2[64 * q:64 * q + 64, :] = wim_s[q::2, :].T
        in_maps.append({
            "u": np.ascontiguousarray(u[:, :, h0:h0 + HS], dtype=f32),
            "D": np.ascontiguousarray(D[h0:h0 + HS], dtype=f32).reshape(1, HS),
            "logdt": np.ascontiguousarray(log_dt[h0:h0 + HS], dtype=f32).reshape(1, HS),
            "Wre2": wre2,
            "Wim2": wim2,
            "Lre": np.ascontiguousarray(Lambda_re, dtype=f32).reshape(1, N),
            "Lim": np.ascontiguousarray(Lambda_im, dtype=f32).reshape(1, N),
            "CF": _prog.CF,
            "AI": _prog.AI,
            "SGN": _prog.SGN,
            "SEL": _prog.SEL,
        })
    return in_maps


LAST_RESULTS = None


def kernel(u, D, Lambda_re, Lambda_im, log_dt, W_re, W_im):
    global LAST_RESULTS
    from concourse.bass_utils import run_bass_kernel_spmd
    in_maps = make_in_maps(u, D, log_dt, W_re, W_im, Lambda_re, Lambda_im)
    res = run_bass_kernel_spmd(_prog.nc, in_maps, core_ids=list(range(NCORES)))
    LAST_RESULTS = res
    y = np.concatenate([res.results[c]["y"] for c in range(NCORES)], axis=2)
    return y.astype(np.float32)
